# revision 6
# baseline (speedup 1.0000x reference)
"""Causal single-head attention (B=8, L=2048, D=1024, H=64) on 8 trn2 NeuronCores.

Strategy: data-parallel over batch — core b handles batch element b.
Per core (flash-attention style, S^T layout, no on-device input transposes —
the host supplies X^T pre-blocked so the contraction dim (d_model) lands on
partitions and every DMA reads long contiguous runs):

  software pipeline over q-blocks j (QB columns of Q^T):
    stream x column-blocks from DRAM; projections column-packed on the PE
    (Q cols 0-63 / K cols 64-127; V even-d-tiles cols 0-63 / odd 64-127
    with a DVE merge-add); V^T -> Vn via a small PE matmul against a
    scaled identity (folds the fp8 weight-scale correction in for free).
    attention(j): per k-tile pair (row-packed, concurrent since K=H=64):
      S^T = K^T.T @ Q^T ; E = exp(S^T/8) on ACT ; diagonal masked by DVE ;
      acc[65,QB] += Vn.T @ E (row 64 = softmax denominator).
    finalize: PE-transpose acc, DVE reciprocal+scale, bf16 DMA store.

Precision plan (validated against the reference in fp-exact numpy sim,
rel-err 2.95e-3 == the all-bf16 kernel): softmax rows with few keys are
the only fp8-sensitive outputs, and they all live in the first 128 rows.
So rows 0-127 of xq/xk/xv ship bf16; rows 128+ ship fp8e4. fp8 weights are
host-prescaled by 32 (avoids e4m3 subnormals); the 1/32 un-scale folds
into the PSUM->SBUF copies (DVE tensor_scalar) and the Vn identity.

Engine budget (from ntff profiles): exp runs ONLY on ACT, so ACT carries
nothing else — projection copies go to DVE, masks are generated on-chip by
gpsimd. Input loads own the sync (qSP) HWDGE ring in need-order; weights,
the deferred V loads (issued behind early exps so they cannot steal HBM
bandwidth from the critical block-0 stream) and output stores ride the
scalar (qAct) ring.
"""
import os
import sys

sys.path.insert(0, "/opt/trn_rl_repo")

import ml_dtypes
import numpy as np

import concourse.bass as bass
import concourse.tile as tile
from concourse import mybir
from concourse.bass_utils import run_bass_kernel_spmd
from concourse.masks import make_identity
from bass_rust import ScopedClock, SyncInfo

B, L, D, H = 8, 2048, 1024, 64
QB = 512                 # q-block width
NQ = L // QB             # q-blocks per core
KT = QB // 128           # 128-k-tiles per q-block
ND = D // 128            # d_model tiles
NCORES = 8
RB = 128                 # leading rows shipped in bf16 (few-key rows)
RF = QB - RB             # fp8 tail of block 0
WS = 32.0                # fp8 weight prescale (un-scaled on-chip)

f32d = mybir.dt.float32
f8 = mybir.dt.float8e4
bf16 = mybir.dt.bfloat16
_NPF8 = mybir.dt.np(f8)
_NPBF = ml_dtypes.bfloat16

# ---------------------------------------------------------------------------
# Workarounds for the container's walrus: max ONE sync-wait per instruction.
_WAIT_CAP = 1


def _patched_drain_and_barrier(self, tick_clock, wait_clock):
    drain_inst = self.nc.sync.drain()
    wait_clock.add_sem_waits(
        drain_inst.ins, ScopedClock({None: tick_clock.global_clock})
    )
    conds = list(drain_inst.ins.sync_info.on_wait)
    if len(conds) > _WAIT_CAP:
        drain_inst.ins.sync_info.on_wait.clear()
        drain_inst.ins.sync_info.on_wait.append(conds[0])
        for c in conds[1:]:
            extra = self.nc.sync.drain()
            if extra.ins.sync_info is None:
                extra.ins.sync_info = SyncInfo(on_wait=[c], on_update=[])
            else:
                extra.ins.sync_info.on_wait.append(c)
    if os.environ.get("ATT_FULL_TAIL", "0") == "1":
        self.nc.all_engine_barrier()
        assert self.sems is not None
        popped = self.nc._tile_sem_poison_stack.pop()
        assert popped is self._sem_poison
        self.nc.clear_and_free_semaphores(list(self.sems.allocated().values()))
        self.nc.all_engine_barrier()
    else:
        # the NEFF preamble re-zeroes all semaphores at the start of every
        # execution, so the expensive tail butterfly + per-sem clears are
        # redundant; just pop the bookkeeping.
        assert self.sems is not None
        popped = self.nc._tile_sem_poison_stack.pop()
        assert popped is self._sem_poison


tile.TileContext._drain_and_barrier = _patched_drain_and_barrier


def _split_excess_waits(nc):
    """Hoist overflow sem-waits onto same-engine NOPs inserted just before the
    offending instruction (engines execute their stream in order)."""
    for bb in nc.main_func.blocks:
        il = bb.instructions
        i = 0
        while i < len(il):
            ins = il[i]
            si = ins.sync_info
            if si is not None and si.on_wait and len(si.on_wait) > _WAIT_CAP:
                conds = list(si.on_wait)
                keep = conds[-_WAIT_CAP:]
                pre = conds[:-_WAIT_CAP]
                si.on_wait.clear()
                si.on_wait.extend(keep)
                nops = []
                for j in range(0, len(pre), _WAIT_CAP):
                    nop = nc.engines[ins.engine].nop().ins
                    for srcbb in nc.main_func.blocks:
                        sl = srcbb.instructions
                        if sl and sl[-1].name == nop.name:
                            sl.pop()
                            break
                    nop.sync_info = SyncInfo(
                        on_wait=list(pre[j : j + _WAIT_CAP]), on_update=[]
                    )
                    nops.append(nop)
                for k, nop in enumerate(nops):
                    il.insert(i + k, nop)
                i += len(nops)
            i += 1


# ---------------------------------------------------------------------------
def _build_nc():
    f32 = f32d
    nc = bass.Bass("TRN2", target_bir_lowering=False)

    # pre-blocked: x[p, a, n] = X^T[a*128+p, col_base+n] — per (partition)
    # the ND*ncols elements are contiguous, so DMA descriptors are long runs.
    xq0b = nc.dram_tensor("xq0b", [128, ND, RB], bf16, kind="ExternalInput")
    xk0b = nc.dram_tensor("xk0b", [128, ND, RB], bf16, kind="ExternalInput")
    xv0b = nc.dram_tensor("xv0b", [128, ND, RB], bf16, kind="ExternalInput")
    xq0f = nc.dram_tensor("xq0f", [128, ND, RF], f8, kind="ExternalInput")
    xk0f = nc.dram_tensor("xk0f", [128, ND, RF], f8, kind="ExternalInput")
    xv0f = nc.dram_tensor("xv0f", [128, ND, RF], f8, kind="ExternalInput")
    xq8 = nc.dram_tensor("xq8", [NQ - 1, 128, ND, QB], f8, kind="ExternalInput")
    xk8 = nc.dram_tensor("xk8", [NQ - 1, 128, ND, QB], f8, kind="ExternalInput")
    xv8 = nc.dram_tensor("xv8", [NQ - 1, 128, ND, QB], f8, kind="ExternalInput")
    # pre-swizzled: w[p, a, h] = W[a*128+p, h]; fp8 copies carry the WS scale
    wq = nc.dram_tensor("wq", [128, ND, H], bf16, kind="ExternalInput")
    wk = nc.dram_tensor("wk", [128, ND, H], bf16, kind="ExternalInput")
    wv = nc.dram_tensor("wv", [128, ND, H], bf16, kind="ExternalInput")
    wq8 = nc.dram_tensor("wq8", [128, ND, H], f8, kind="ExternalInput")
    wk8 = nc.dram_tensor("wk8", [128, ND, H], f8, kind="ExternalInput")
    wv8 = nc.dram_tensor("wv8", [128, ND, H], f8, kind="ExternalInput")
    outd = nc.dram_tensor("out", [NQ, 128, KT, H], bf16, kind="ExternalOutput")

    with tile.TileContext(nc) as tc:
        with (
            tc.tile_pool(name="consts", bufs=1) as consts,
            tc.tile_pool(name="xin", bufs=3) as xin,
            tc.tile_pool(name="work", bufs=6) as work,
            tc.tile_pool(name="ps_s", bufs=2, space="PSUM") as ps_s,
            tc.tile_pool(name="ps_acc", bufs=2, space="PSUM") as ps_acc,
            tc.tile_pool(name="ps_misc", bufs=2, space="PSUM") as ps_misc,
        ):
            # ---- block-0 input tiles (single-use, so they live in consts)
            xq0b_t = consts.tile([128, ND, RB], bf16)
            xk0b_t = consts.tile([128, ND, RB], bf16)
            xv0b_t = consts.tile([128, ND, RB], bf16)
            xq0f_t = consts.tile([128, ND, RF], f8)
            xk0f_t = consts.tile([128, ND, RF], f8)
            xv0f_t = consts.tile([128, ND, RF], f8)
            # need-ordered on the sync ring: q then k then v
            nc.sync.dma_start(out=xq0b_t, in_=xq0b[:, :, :])
            nc.sync.dma_start(out=xq0f_t, in_=xq0f[:, :, :])
            nc.sync.dma_start(out=xk0b_t, in_=xk0b[:, :, :])
            nc.sync.dma_start(out=xk0f_t, in_=xk0f[:, :, :])
            nc.sync.dma_start(out=xv0b_t, in_=xv0b[:, :, :])
            nc.sync.dma_start(out=xv0f_t, in_=xv0f[:, :, :])

            # weights on the scalar ring (small, early)
            wq_t = consts.tile([128, ND, H], bf16)
            wk_t = consts.tile([128, ND, H], bf16)
            wv_t = consts.tile([128, ND, H], bf16)
            wq8_t = consts.tile([128, ND, H], f8)
            wk8_t = consts.tile([128, ND, H], f8)
            wv8_t = consts.tile([128, ND, H], f8)
            nc.scalar.dma_start(out=wq_t, in_=wq[:, :, :])
            nc.scalar.dma_start(out=wk_t, in_=wk[:, :, :])
            nc.scalar.dma_start(out=wv_t, in_=wv[:, :, :])
            nc.scalar.dma_start(out=wq8_t, in_=wq8[:, :, :])
            nc.scalar.dma_start(out=wk8_t, in_=wk8[:, :, :])
            nc.scalar.dma_start(out=wv8_t, in_=wv8[:, :, :])

            ident = consts.tile([128, 128], f32)
            make_identity(nc, ident)
            # bf16 identities for the Vn transpose-matmuls: diag 1 (true
            # scale) and diag 1/WS (un-scales fp8-weight projections)
            identb = consts.tile([128, 128], bf16)
            identbs = consts.tile([128, 128], bf16)
            nc.vector.tensor_copy(identb, ident)
            nc.vector.tensor_scalar_mul(identbs, ident, 1.0 / WS)

            # QTd: Q^T duplicated on both partition halves (rhs for the two
            # row-groups of the packed S matmuls). KTt2: K^T k-tiles stored
            # even-on-partitions-0:63 / odd-on-64:127, pair t at cols t*128.
            QTd = consts.tile([128, L], bf16)
            KTt2 = consts.tile([128, L // 2], bf16)
            Vn = consts.tile([128, L // 128, H + 1], bf16)
            ones_sb = consts.tile([128, L // 128], bf16)
            nc.gpsimd.memset(ones_sb, 1.0)
            nc.vector.tensor_copy(
                Vn[:, :, H : H + 1], ones_sb.rearrange("p (a o) -> p a o", o=1)
            )
            # diagonal causal masks, on-chip: maskt[p, d, f] = (p+128d <= f)
            maskt = consts.tile([128, KT, QB], bf16)
            nc.gpsimd.memset(maskt, 1.0)
            for dd in range(KT):
                nc.gpsimd.affine_select(
                    out=maskt[:, dd, :],
                    in_=maskt[:, dd, :],
                    compare_op=mybir.AluOpType.is_ge,
                    fill=0.0,
                    base=-128 * dd,
                    pattern=[[1, QB]],
                    channel_multiplier=-1,
                )

            def issue_block_dma(j):
                """Blocks 1..NQ-1: fp8 q/k on the sync ring; v is issued
                separately (deferred, scalar ring) via issue_v_dma."""
                xq_b = xin.tile([128, ND, QB], f8, tag="xq")
                xk_b = xin.tile([128, ND, QB], f8, tag="xk")
                nc.sync.dma_start(out=xq_b, in_=xq8[j - 1, :, :, :])
                nc.sync.dma_start(out=xk_b, in_=xk8[j - 1, :, :, :])
                return xq_b, xk_b

            def issue_v_dma(j):
                xv_b = xin.tile([128, ND, QB], f8, tag="xv")
                nc.scalar.dma_start(out=xv_b, in_=xv8[j - 1, :, :, :])
                return xv_b

            def proj_qk0():
                """Block 0: bf16 head columns + fp8 tail columns."""
                psp = ps_misc.tile([128, QB], f32, tag="misc")
                for kb in range(ND):
                    nc.tensor.matmul(
                        psp[0:64, 0:RB], wq_t[:, kb, :], xq0b_t[:, kb, :],
                        start=(kb == 0), stop=(kb == ND - 1),
                        tile_position=(0, 0),
                    )
                    nc.tensor.matmul(
                        psp[64:128, 0:RB], wk_t[:, kb, :], xk0b_t[:, kb, :],
                        start=(kb == 0), stop=(kb == ND - 1),
                        tile_position=(0, 64),
                    )
                for kb in range(ND):
                    nc.tensor.matmul(
                        psp[0:64, RB:QB], wq8_t[:, kb, :], xq0f_t[:, kb, :],
                        start=(kb == 0), stop=(kb == ND - 1),
                        tile_position=(0, 0),
                    )
                    nc.tensor.matmul(
                        psp[64:128, RB:QB], wk8_t[:, kb, :], xk0f_t[:, kb, :],
                        start=(kb == 0), stop=(kb == ND - 1),
                        tile_position=(0, 64),
                    )
                # Q^T to both partition halves; un-scale the fp8 columns
                for half in (slice(0, 64), slice(64, 128)):
                    nc.vector.tensor_copy(QTd[half, 0:RB], psp[0:64, 0:RB])
                    nc.vector.tensor_scalar_mul(
                        QTd[half, RB:QB], psp[0:64, RB:QB], 1.0 / WS
                    )
                # K^T k-tiles by parity: tile0 true-scale, tiles 1-3 scaled
                kview = psp[64:128, :].rearrange("p (a c) -> p a c", c=128)
                nc.vector.tensor_copy(KTt2[0:64, 0:128], kview[:, 0, :])
                nc.vector.tensor_scalar_mul(
                    KTt2[0:64, 128:256], kview[:, 2, :], 1.0 / WS
                )
                nc.vector.tensor_scalar_mul(
                    KTt2[64:128, 0:256], kview[:, 1::2, :], 1.0 / WS
                )

            def proj_qk(j, xq_b, xk_b):
                """Blocks 1+: fp8, column-packed Q cols 0-63 / K cols 64-127."""
                qs = bass.ts(j, QB)
                psp = ps_misc.tile([128, QB], f32, tag="misc")
                for kb in range(ND):
                    nc.tensor.matmul(
                        psp[0:64, :], wq8_t[:, kb, :], xq_b[:, kb, :],
                        start=(kb == 0), stop=(kb == ND - 1),
                        tile_position=(0, 0),
                    )
                    nc.tensor.matmul(
                        psp[64:128, :], wk8_t[:, kb, :], xk_b[:, kb, :],
                        start=(kb == 0), stop=(kb == ND - 1),
                        tile_position=(0, 64),
                    )
                nc.vector.tensor_scalar_mul(QTd[0:64, qs], psp[0:64, :], 1.0 / WS)
                nc.vector.tensor_scalar_mul(
                    QTd[64:128, qs], psp[0:64, :], 1.0 / WS
                )
                pcols = bass.ds(2 * j * 128, 256)
                kview = psp[64:128, :].rearrange("p (a c) -> p a c", c=128)
                nc.vector.tensor_scalar_mul(
                    KTt2[0:64, pcols], kview[:, 0::2, :], 1.0 / WS
                )
                nc.vector.tensor_scalar_mul(
                    KTt2[64:128, pcols], kview[:, 1::2, :], 1.0 / WS
                )

            def _v_common(j, psp):
                """Merge packed V halves, transpose to Vn (identity matmul
                un-scales the fp8-weight tiles)."""
                # DVE can read only one PSUM operand per instruction: stage
                # the odd-half through SBUF, then add.
                vodd = work.tile([64, QB], f32d, tag="vodd")
                nc.vector.tensor_copy(vodd, psp[64:128, :])
                vts = work.tile([64, QB], bf16, tag="vts")
                nc.vector.tensor_add(vts, psp[0:64, :], vodd)
                for t4 in range(KT):
                    kt_idx = j * KT + t4
                    idm = identb if (j == 0 and t4 == 0) else identbs
                    pst = ps_misc.tile([128, H + 1], f32, tag="misc")
                    nc.tensor.matmul(
                        pst[:, 0:H],
                        vts[:, bass.ts(t4, 128)],
                        idm[0:64, 0:H],
                        start=True,
                        stop=True,
                    )
                    nc.vector.tensor_copy(Vn[:, kt_idx, 0:H], pst[:, 0:H])

            def proj_v0():
                psp = ps_misc.tile([128, QB], f32, tag="misc")
                for kb in range(0, ND, 2):
                    nc.tensor.matmul(
                        psp[0:64, 0:RB], wv_t[:, kb, :], xv0b_t[:, kb, :],
                        start=(kb == 0), stop=(kb == ND - 2),
                        tile_position=(0, 0),
                    )
                    nc.tensor.matmul(
                        psp[64:128, 0:RB], wv_t[:, kb + 1, :],
                        xv0b_t[:, kb + 1, :],
                        start=(kb == 0), stop=(kb == ND - 2),
                        tile_position=(0, 64),
                    )
                for kb in range(0, ND, 2):
                    nc.tensor.matmul(
                        psp[0:64, RB:QB], wv8_t[:, kb, :], xv0f_t[:, kb, :],
                        start=(kb == 0), stop=(kb == ND - 2),
                        tile_position=(0, 0),
                    )
                    nc.tensor.matmul(
                        psp[64:128, RB:QB], wv8_t[:, kb + 1, :],
                        xv0f_t[:, kb + 1, :],
                        start=(kb == 0), stop=(kb == ND - 2),
                        tile_position=(0, 64),
                    )
                _v_common(0, psp)

            def proj_v(j, x_b):
                psp = ps_misc.tile([128, QB], f32, tag="misc")
                for kb in range(0, ND, 2):
                    nc.tensor.matmul(
                        psp[0:64, :], wv8_t[:, kb, :], x_b[:, kb, :],
                        start=(kb == 0), stop=(kb == ND - 2),
                        tile_position=(0, 0),
                    )
                    nc.tensor.matmul(
                        psp[64:128, :], wv8_t[:, kb + 1, :], x_b[:, kb + 1, :],
                        start=(kb == 0), stop=(kb == ND - 2),
                        tile_position=(0, 64),
                    )
                _v_common(j, psp)

            def attention_pair(j, tp, nkt, acc):
                qs = bass.ts(j, QB)
                pss = ps_s.tile([128, 2, QB], f32, tag="s")
                exps = work.tile([128, 2, QB], bf16, tag="exps")
                # row-packed: even k-tile on PE rows 0-63, odd on 64-127,
                # running concurrently (K=64 each)
                nc.tensor.matmul(
                    pss[:, 0, :],
                    KTt2[0:64, bass.ts(tp, 128)],
                    QTd[0:64, qs],
                    start=True,
                    stop=True,
                    tile_position=(0, 0),
                )
                nc.tensor.matmul(
                    pss[:, 1, :],
                    KTt2[64:128, bass.ts(tp, 128)],
                    QTd[64:128, qs],
                    start=True,
                    stop=True,
                    tile_position=(64, 0),
                )
                nc.scalar.activation(
                    exps, pss, mybir.ActivationFunctionType.Exp, scale=0.125
                )
                for i in range(2):
                    t = 2 * tp + i
                    dt_diag = 2 * tp + i - j * KT
                    if dt_diag >= 0:
                        nc.vector.tensor_mul(
                            exps[:, i, :],
                            exps[:, i, :],
                            maskt[:, dt_diag, :],
                        )
                    nc.tensor.matmul(
                        acc,
                        Vn[:, t, :],
                        exps[:, i, :],
                        start=(t == 0),
                        stop=(t == nkt - 1),
                    )

            def finalize_block(j, acc):
                oT = work.tile([H + 1, QB], f32, tag="oT")
                nc.vector.tensor_copy(oT, acc)
                obuf = work.tile([128, KT, H], bf16, tag="obuf")
                for t4 in range(KT):
                    pso = ps_misc.tile([128, H + 1], f32, tag="misc")
                    nc.tensor.transpose(
                        pso, oT[:, bass.ts(t4, 128)], ident[0 : H + 1, 0 : H + 1]
                    )
                    rcp = work.tile([128, 1], f32, tag="rcp")
                    nc.vector.reciprocal(rcp, pso[:, H : H + 1])
                    nc.vector.tensor_scalar_mul(obuf[:, t4, :], pso[:, 0:H], rcp)
                nc.scalar.dma_start(out=outd[j, :, :, :], in_=obuf)

            # ---- software pipeline.
            # warm the PE (HAM un-throttles after ~3.4us of sustained work)
            # while the first activation block is still in flight
            for _ in range(18):
                dum = ps_misc.tile([128, H + 1], f32, tag="misc")
                nc.tensor.matmul(
                    dum, ident, ident[:, 0 : H + 1], start=True, stop=True
                )
            pending = {}
            for jj in range(1, NQ):
                pending[jj] = issue_block_dma(jj)
            pending_v = {}
            proj_qk0()
            proj_v0()
            # deferred V loads: issue on the scalar ring behind an early exp
            # so they never compete with the block-0 / qk streams
            v_defer = {(0, 0): 1, (0, 1): 2, (1, 1): 3}
            for j in range(NQ):
                units = []
                if j + 1 < NQ:
                    nxq, nxk = pending.pop(j + 1)
                    units = [lambda: proj_qk(j + 1, nxq, nxk)]
                acc = ps_acc.tile([H + 1, QB], f32)
                nkt = (j + 1) * KT
                npairs = nkt // 2
                # schedule: proj_v(j) right after pair 1 (its Vn tiles are
                # first consumed at pair 2j); proj_qk(j+1) mid-block
                slots = {}
                if j + 1 < NQ:
                    slots.setdefault(min(npairs - 1, max(1, npairs // 2)), []).extend(
                        units
                    )
                if j in pending_v:
                    slots.setdefault(1, []).insert(0, (lambda jj=j: proj_v(jj, pending_v.pop(jj))))
                for tp in range(npairs):
                    attention_pair(j, tp, nkt, acc)
                    jv = v_defer.get((j, tp))
                    if jv is not None:
                        pending_v[jv] = issue_v_dma(jv)
                    for u in slots.get(tp, []):
                        u()
                finalize_block(j, acc)

    _split_excess_waits(nc)
    return nc


_NC = None


def _get_nc():
    global _NC
    if _NC is None:
        _NC = _build_nc()
    return _NC


def _block_rows(x, r0, r1, npdt):
    """rows [r0:r1) of [L, D] -> [128, ND, r1-r0] pre-blocked X^T."""
    return np.ascontiguousarray(
        np.asarray(x[r0:r1], np.float32)
        .reshape(r1 - r0, ND, 128)
        .transpose(2, 1, 0)
        .astype(npdt)
    )


def _block_x8(x):
    """rows [QB:] of [L, D] -> [NQ-1, 128, ND, QB] fp8 pre-blocked X^T."""
    return np.ascontiguousarray(
        np.asarray(x[QB:], np.float32)
        .reshape(NQ - 1, QB, ND, 128)
        .transpose(0, 3, 2, 1)
        .astype(_NPF8)
    )


def _swizzle_w(w, scale, npdt):
    """[D, H] -> [128, ND, H]: w[p, a, h] = scale * W[a*128+p, h]."""
    return np.ascontiguousarray(
        (np.asarray(w, np.float32) * scale)
        .reshape(ND, 128, H)
        .transpose(1, 0, 2)
        .astype(npdt)
    )


def make_in_maps(inputs):
    """Build per-core in_maps from a reference-style inputs dict."""
    shared = {
        "wq": _swizzle_w(inputs["Wq"], 1.0, _NPBF),
        "wk": _swizzle_w(inputs["Wk"], 1.0, _NPBF),
        "wv": _swizzle_w(inputs["Wv"], 1.0, _NPBF),
        "wq8": _swizzle_w(inputs["Wq"], WS, _NPF8),
        "wk8": _swizzle_w(inputs["Wk"], WS, _NPF8),
        "wv8": _swizzle_w(inputs["Wv"], WS, _NPF8),
    }
    maps = []
    for b in range(NCORES):
        m = dict(shared)
        for name, key in (("xq", "idx_q"), ("xk", "idx_k"), ("xv", "idx_v")):
            x = inputs[key][b]
            m[name + "0b"] = _block_rows(x, 0, RB, _NPBF)
            m[name + "0f"] = _block_rows(x, RB, QB, _NPF8)
            m[name + "8"] = _block_x8(x)
        maps.append(m)
    return maps


def kernel(idx_k, idx_q, idx_v, msk, Wk, Wq, Wv, **_unused):
    in_maps = make_in_maps(
        {
            "idx_k": idx_k,
            "idx_q": idx_q,
            "idx_v": idx_v,
            "Wk": Wk,
            "Wq": Wq,
            "Wv": Wv,
        }
    )
    nc = _get_nc()
    res = run_bass_kernel_spmd(nc, in_maps, core_ids=list(range(NCORES)))
    return np.stack(
        [
            res.results[b]["out"]
            .astype(np.float32)
            .transpose(0, 2, 1, 3)
            .reshape(L, H)
            for b in range(NCORES)
        ],
        axis=0,
    )


def run_traced(in_maps, tmpdir="/tmp/att_trace", trace_cores=None):
    """Test-harness helper: run with NTFF tracing, return BassKernelResults."""
    import os
    import shutil

    shutil.rmtree(tmpdir, ignore_errors=True)
    os.makedirs(tmpdir)
    return run_bass_kernel_spmd(
        _get_nc(),
        in_maps,
        core_ids=list(range(NCORES)),
        trace=True,
        tmpdir=tmpdir,
        trace_cores=trace_cores,
    )
